# revision 1
# baseline (speedup 1.0000x reference)
"""Multi-head GAT layer on 8 Trainium2 NeuronCores (Bass/Tile).

Strategy
--------
Edges are sorted by src (= softmax segment = output row) on the host and
sharded across the 8 cores at node boundaries, so every segment lives
entirely on one core and no cross-core reduction is needed.

Per core (SPMD, one program, uniform shapes):
  Phase W: compute the full tables  Wh = h @ W  (all heads, [N,1024] bf16)
           and scores s_src/s_dst = Wh @ a     ([N,16] fp32, stored as
           256-byte rows for gather) with a tiled PE matmul, and write them
           to core-local HBM.
  Phase A: stream this core's edges in chunks:
           - dma_gather score rows by src and by dst -> e = lrelu(s+s)
           - exp via ACT
           - dma_gather message rows G = Wh[dst]  (bf16, 2KB/row)
           - scale G in place by exp(e) per head (DVE, broadcast AP)
           - per 128-edge tile, build the src one-hot S01 (iota==srcloc)
             and matmul  psum[nodes,1032] += S01^T @ [G | exp]  -- the
             last 8 columns accumulate the softmax denominators.
           - per node window, normalize psum rows by 1/denominator and
             DMA out.
Host reassembles the per-core node slices into the [N, H*D] output.
"""

import sys

sys.path.insert(0, "/opt/trn_rl_repo")

import numpy as np
import ml_dtypes

# ---------------------------------------------------------------------------
# Patch: this environment's walrus codegen supports at most ONE sem-wait per
# instruction. Split multi-wait instructions into single-wait nop chains.
# ---------------------------------------------------------------------------
import concourse.tile as tile_mod
import concourse.mybir as mybir
from concourse.vector_clock import ScopedClock

_MAX_WAITS = 1

_orig_add_instruction = tile_mod.TileContext._add_instruction


def _make_wait_nop(nc, engine, waits):
    nop = mybir.InstNoOp(name=nc.get_next_instruction_name(), ins=[], outs=[])
    nop.engine = engine
    nop.sync_info = mybir.SyncInfo(on_wait=list(waits), on_update=[])
    return nop


def _patched_add_instruction(self, inst):
    si = getattr(inst, "sync_info", None)
    if si is not None and len(si.on_wait) > _MAX_WAITS:
        waits = list(si.on_wait)
        for i in range(0, len(waits) - _MAX_WAITS, _MAX_WAITS):
            _orig_add_instruction(
                self, _make_wait_nop(self.nc, inst.engine, waits[i : i + _MAX_WAITS])
            )
        si.on_wait = waits[len(waits) - _MAX_WAITS :]
        inst.sync_info = si
    _orig_add_instruction(self, inst)


tile_mod.TileContext._add_instruction = _patched_add_instruction


def _patched_drain_and_barrier(self, tick_clock, wait_clock):
    nc = self.nc
    probe = nc.sync.nop(nofuse=True).ins
    wait_clock.add_sem_waits(probe, ScopedClock({None: tick_clock.global_clock}))
    si = probe.sync_info
    waits = list(si.on_wait) if si else []
    if si and len(waits) > _MAX_WAITS:
        si.on_wait = waits[:_MAX_WAITS]
        probe.sync_info = si
        for i in range(_MAX_WAITS, len(waits), _MAX_WAITS):
            n = nc.sync.nop(nofuse=True).ins
            nsi = n.sync_info
            if nsi is None:
                nsi = mybir.SyncInfo(on_wait=[], on_update=[])
            nsi.on_wait = waits[i : i + _MAX_WAITS]
            n.sync_info = nsi
    nc.sync.drain()
    nc.all_engine_barrier()
    assert self.sems is not None
    popped = nc._tile_sem_poison_stack.pop()
    assert popped is self._sem_poison
    nc.clear_and_free_semaphores(list(self.sems.allocated().values()))
    nc.all_engine_barrier()


tile_mod.TileContext._drain_and_barrier = _patched_drain_and_barrier

import concourse.bass as bass
import concourse.tile as tile
from concourse.bass_utils import run_bass_kernel_spmd
from concourse import library_config
from concourse.library_overlay import lower_extended_insts

NCORES = 8
P = 128
ALPHA = 0.2
F32 = mybir.dt.float32
BF16 = mybir.dt.bfloat16
I16 = mybir.dt.int16
PAD_SRCLOC = 300.0  # sentinel: never matches iota column 0..127


def _wrap_idx(idx_list):
    """[E] int -> [128, E/16] int16 wrap layout for dma_gather."""
    a = idx_list.astype(np.int16).reshape(-1, 16).T  # [16, E/16]
    return np.ascontiguousarray(np.tile(a, (8, 1)))  # [128, E/16]


def _prep_edges(src, dst, N, E):
    """Sort by src, shard at node boundaries, build uniform window/tile grid."""
    perm = np.argsort(src, kind="stable")
    ssrc = src[perm]
    sdst = dst[perm]
    counts = np.bincount(src, minlength=N)
    cum = np.concatenate([[0], np.cumsum(counts)])  # [N+1]

    # core cuts at node granularity, balanced by edge count
    targets = (np.arange(1, NCORES) * E) // NCORES
    cut_nodes = np.searchsorted(cum, targets, side="left")
    node_bounds = np.concatenate([[0], cut_nodes, [N]]).astype(np.int64)
    node_bounds = np.maximum.accumulate(node_bounds)

    # per (core, window) edge counts; windows are aligned 128-node blocks
    # within the core's node range
    per_core = []
    max_nwin = 0
    max_wtiles = 0
    for c in range(NCORES):
        nlo, nhi = int(node_bounds[c]), int(node_bounds[c + 1])
        width = nhi - nlo
        nwin = max(1, -(-width // P))
        max_nwin = max(max_nwin, nwin)
        wedges = []
        for w in range(nwin):
            a = nlo + w * P
            b = min(a + P, nhi)
            cnt = int(cum[b] - cum[a])
            wedges.append(cnt)
            max_wtiles = max(max_wtiles, -(-cnt // P))
        per_core.append((nlo, nhi, nwin, wedges))

    # uniform tiles-per-window, rounded up to a whole number of gather
    # chunks of at most CH_TARGET tiles (large single gathers hang the
    # SWDGE path -- empirically <=5 tiles/2176 idx is safe)
    import os
    ch_target = int(os.environ.get("KERNEL_CH", "5"))
    cpw = -(-max_wtiles // ch_target)
    wpt = cpw * ch_target
    nwin = max_nwin
    nt = nwin * wpt
    ec = nt * P

    cores = []
    for c in range(NCORES):
        nlo, nhi, c_nwin, wedges = per_core[c]
        gdst = np.zeros(ec, dtype=np.int64)
        gsrc = np.zeros(ec, dtype=np.int64)
        srcloc = np.full(ec, PAD_SRCLOC, dtype=np.float32)
        for w in range(c_nwin):
            a = nlo + w * P
            b = min(a + P, nhi)
            e0, e1 = int(cum[a]), int(cum[b])
            n = e1 - e0
            if n == 0:
                continue
            base = w * wpt * P
            gdst[base : base + n] = sdst[e0:e1]
            gsrc[base : base + n] = ssrc[e0:e1]
            srcloc[base : base + n] = (ssrc[e0:e1] - a).astype(np.float32)
        cores.append(
            dict(
                nlo=nlo,
                nhi=nhi,
                gdst_w=_wrap_idx(gdst),
                gsrc_w=_wrap_idx(gsrc),
                srcloc2d=np.ascontiguousarray(
                    srcloc.reshape(nt, P).T
                ),  # [128, NT] fp32
            )
        )
    return cores, nwin, wpt, nt, ec


def build(h, edge_idx, W, a):
    """Trace the SPMD program; returns (nc, in_maps, assemble_fn)."""
    N, Din = h.shape
    H, _, Dout = W.shape
    E = edge_idx.shape[1]
    HD = H * Dout  # 1024
    KC = Din // P  # 4 contraction chunks
    SC = 2 * H  # 16 score columns
    WSC = HD + SC  # 1040 matmul output columns
    SCROW = 64  # score-table row, 64 f32 = 256B
    NPAD = -(-N // P) * P
    NTBL = NPAD // P  # table tiles

    src = edge_idx[0].astype(np.int64)
    dst = edge_idx[1].astype(np.int64)
    cores, NWIN, WPT, NT, EC = _prep_edges(src, dst, N, E)
    import os
    CH = int(os.environ.get("KERNEL_CH", "5"))  # tiles per gather chunk
    CPW = WPT // CH  # chunks per window
    NCHUNK = NT // CH

    # ---- host-side weight/feature packing (bf16) ----
    a_src = a[:, :Dout, 0]  # [H, Dout]
    a_dst = a[:, Dout:, 0]
    Wall = np.concatenate([W[hh] for hh in range(H)], axis=1)  # [Din, HD]
    Wsrc = np.stack([W[hh] @ a_src[hh] for hh in range(H)], axis=1)  # [Din, H]
    Wdst = np.stack([W[hh] @ a_dst[hh] for hh in range(H)], axis=1)
    Wfull = np.concatenate([Wall, Wsrc, Wdst], axis=1).astype(np.float32)
    # SBUF layout [128, KC, WSC]: [p, kc, j] = Wfull[kc*128+p, j]
    Wsb = np.ascontiguousarray(
        Wfull.reshape(KC, P, WSC).transpose(1, 0, 2)
    ).astype(ml_dtypes.bfloat16)

    hpad = np.zeros((NPAD, Din), dtype=np.float32)
    hpad[:N] = h
    hT = hpad.T  # [Din, NPAD]
    # [NTBL, 128, KC*128]: [nt, p, kc*128+c] = hT[kc*128+p, nt*128+c]
    hTb = np.ascontiguousarray(
        hT.reshape(KC, P, NTBL, P).transpose(2, 1, 0, 3).reshape(NTBL, P, KC * P)
    ).astype(ml_dtypes.bfloat16)

    iota = np.broadcast_to(np.arange(P, dtype=np.float32), (P, P)).copy()

    # ---- build the SPMD program ----
    nc = bass.Bass()
    hTb_d = nc.declare_dram_parameter("hTb", [NTBL, P, KC * P], BF16, isOutput=False)
    Wsb_d = nc.declare_dram_parameter("Wsb", [P, KC, WSC], BF16, isOutput=False)
    iota_d = nc.declare_dram_parameter("iota", [P, P], F32, isOutput=False)
    gdst_d = nc.declare_dram_parameter("gdst", [P, EC // 16], I16, isOutput=False)
    gsrc_d = nc.declare_dram_parameter("gsrc", [P, EC // 16], I16, isOutput=False)
    srcloc_d = nc.declare_dram_parameter("srcloc", [P, NT], F32, isOutput=False)
    out_d = nc.declare_dram_parameter("out", [NWIN * P, HD], F32, isOutput=True)

    whtab = nc.dram_tensor("whtab", [NPAD, HD], BF16)
    sctab = nc.dram_tensor("sctab", [NPAD, SCROW], F32)

    nc.gpsimd.load_library(library_config.mlp)

    with tile.TileContext(nc) as tc:
        with tc.tile_pool(name="consts", bufs=1) as cp:
            iota_t = cp.tile([P, P], F32)
            nc.sync.dma_start(iota_t[:], iota_d[:])
            wsb_t = cp.tile([P, KC, WSC], BF16)
            nc.sync.dma_start(wsb_t[:], Wsb_d[:])
            gdst_t = cp.tile([P, EC // 16], I16)
            nc.sync.dma_start(gdst_t[:], gdst_d[:])
            gsrc_t = cp.tile([P, EC // 16], I16)
            nc.sync.dma_start(gsrc_t[:], gsrc_d[:])
            srcloc_t = cp.tile([P, NT], F32)
            nc.sync.dma_start(srcloc_t[:], srcloc_d[:])

            # ---- Phase W: tables ----
            with (
                tc.tile_pool(name="psw", bufs=2, space="PSUM") as pw,
                tc.tile_pool(name="sbw", bufs=3) as sw,
            ):
                for ntb in range(NTBL):
                    htt = sw.tile([P, KC * P], BF16, tag="ht")
                    nc.sync.dma_start(htt[:], hTb_d[ntb])
                    ps = pw.tile([P, WSC], F32, tag="psw")
                    for kc in range(KC):
                        lhs = htt[:, kc * P : (kc + 1) * P]
                        first, last = kc == 0, kc == KC - 1
                        nc.tensor.matmul(
                            ps[:, 0:512],
                            lhsT=lhs,
                            rhs=wsb_t[:, kc, 0:512],
                            start=first,
                            stop=last,
                        )
                        nc.tensor.matmul(
                            ps[:, 512:1024],
                            lhsT=lhs,
                            rhs=wsb_t[:, kc, 512:1024],
                            start=first,
                            stop=last,
                        )
                        nc.tensor.matmul(
                            ps[:, 1024:WSC],
                            lhsT=lhs,
                            rhs=wsb_t[:, kc, 1024:WSC],
                            start=first,
                            stop=last,
                        )
                    wht = sw.tile([P, HD], BF16, tag="wht")
                    nc.scalar.activation(
                        wht[:], ps[:, 0:HD], mybir.ActivationFunctionType.Copy
                    )
                    sct = sw.tile([P, SCROW], F32, tag="sct")
                    nc.vector.memset(sct[:, SC:SCROW], 0.0)
                    nc.vector.tensor_copy(sct[:, 0:SC], ps[:, HD:WSC])
                    nc.sync.dma_start(whtab[ntb * P : (ntb + 1) * P, :], wht[:])
                    nc.sync.dma_start(sctab[ntb * P : (ntb + 1) * P, :], sct[:])

            # ---- Phase A: edges ----
            REPEAT = int(os.environ.get("KERNEL_REPEAT", "1"))
            with (
                tc.tile_pool(name="psa", bufs=2, space="PSUM") as pa,
                tc.tile_pool(name="sba", bufs=int(os.environ.get("KERNEL_BUFS", "2"))) as sa,
                tc.tile_pool(name="sbo", bufs=2) as so,
            ):
                ni_reg = nc.gpsimd.to_reg(CH * P)

                def phase_a():
                    for w in range(NWIN):
                        ps = pa.tile([P, HD + H], F32, tag="psa")
                        for half in range(CPW):
                            ch = w * CPW + half
                            i0 = ch * CH * 8  # idx col offset (num/16)
                            ni = CH * P
                            ssc = sa.tile([P, CH, SCROW], F32, tag="ssc")
                            nc.gpsimd.dma_gather(
                                ssc[:], sctab[:], gsrc_t[:, i0 : i0 + CH * 8], ni, ni_reg, SCROW
                            )
                            dsc = sa.tile([P, CH, SCROW], F32, tag="dsc")
                            nc.gpsimd.dma_gather(
                                dsc[:], sctab[:], gdst_t[:, i0 : i0 + CH * 8], ni, ni_reg, SCROW
                            )
                            g = sa.tile([P, CH, HD], BF16, tag="g")
                            nc.gpsimd.dma_gather(
                                g[:], whtab[:], gdst_t[:, i0 : i0 + CH * 8], ni, ni_reg, HD
                            )
                            es = sa.tile([P, CH, H], F32, tag="es")
                            nc.vector.tensor_tensor(
                                out=es[:],
                                in0=ssc[:, :, 0:H],
                                in1=dsc[:, :, H : 2 * H],
                                op=mybir.AluOpType.add,
                            )
                            # leaky relu = max(x, alpha*x)
                            el = sa.tile([P, CH, H], F32, tag="el")
                            nc.vector.tensor_scalar(
                                out=el[:],
                                in0=es[:],
                                scalar1=ALPHA,
                                scalar2=None,
                                op0=mybir.AluOpType.mult,
                            )
                            nc.vector.tensor_tensor(
                                out=el[:], in0=el[:], in1=es[:], op=mybir.AluOpType.max
                            )
                            eb = sa.tile([P, CH, H], BF16, tag="eb")
                            nc.scalar.activation(
                                eb[:], el[:], mybir.ActivationFunctionType.Exp
                            )
                            # scale messages by exp(e) per head, in place
                            for hh in range(H):
                                nc.vector.tensor_tensor(
                                    out=g[:, :, hh * Dout : (hh + 1) * Dout],
                                    in0=g[:, :, hh * Dout : (hh + 1) * Dout],
                                    in1=eb[:, :, hh : hh + 1].to_broadcast([P, CH, Dout]),
                                    op=mybir.AluOpType.mult,
                                )
                            for tl in range(CH):
                                gt = ch * CH + tl
                                s01 = sa.tile([P, P], BF16, tag="s01")
                                nc.vector.tensor_scalar(
                                    out=s01[:],
                                    in0=iota_t[:],
                                    scalar1=srcloc_t[:, gt : gt + 1],
                                    scalar2=None,
                                    op0=mybir.AluOpType.is_equal,
                                )
                                first = gt % WPT == 0
                                last = gt % WPT == WPT - 1
                                nc.tensor.matmul(
                                    ps[:, 0:512],
                                    lhsT=s01[:],
                                    rhs=g[:, tl, 0:512],
                                    start=first,
                                    stop=last,
                                )
                                nc.tensor.matmul(
                                    ps[:, 512:1024],
                                    lhsT=s01[:],
                                    rhs=g[:, tl, 512:1024],
                                    start=first,
                                    stop=last,
                                )
                                nc.tensor.matmul(
                                    ps[:, HD : HD + H],
                                    lhsT=s01[:],
                                    rhs=eb[:, tl, :],
                                    start=first,
                                    stop=last,
                                )
                        # flush window: normalize by softmax denominator
                        rs = so.tile([P, H], F32, tag="rs")
                        nc.vector.tensor_scalar(
                            out=rs[:],
                            in0=ps[:, HD : HD + H],
                            scalar1=1e-30,
                            scalar2=None,
                            op0=mybir.AluOpType.max,
                        )
                        rc = so.tile([P, H], F32, tag="rc")
                        nc.vector.reciprocal(rc[:], rs[:])
                        ot = so.tile([P, HD], F32, tag="ot")
                        for hh in range(H):
                            nc.vector.tensor_scalar(
                                out=ot[:, hh * Dout : (hh + 1) * Dout],
                                in0=ps[:, hh * Dout : (hh + 1) * Dout],
                                scalar1=rc[:, hh : hh + 1],
                                scalar2=None,
                                op0=mybir.AluOpType.mult,
                            )
                        nc.sync.dma_start(out_d[w * P : (w + 1) * P, :], ot[:])

                if REPEAT > 1:
                    with tc.For_i(0, REPEAT, 1):
                        phase_a()
                else:
                    phase_a()

    lower_extended_insts(nc)

    in_maps = []
    for c in range(NCORES):
        in_maps.append(
            {
                "hTb": hTb,
                "Wsb": Wsb,
                "iota": iota,
                "gdst": cores[c]["gdst_w"],
                "gsrc": cores[c]["gsrc_w"],
                "srcloc": cores[c]["srcloc2d"],
            }
        )
    def assemble(results):
        out = np.zeros((N, HD), dtype=np.float32)
        for c in range(NCORES):
            nlo, nhi = cores[c]["nlo"], cores[c]["nhi"]
            o = results[c]["out"]
            width = nhi - nlo
            out[nlo:nhi] = o[:width]
        return out

    return nc, in_maps, assemble


def kernel(h, edge_idx, W, a):
    nc, in_maps, assemble = build(h, edge_idx, W, a)
    res = run_bass_kernel_spmd(nc, in_maps, list(range(NCORES)))
    return assemble(res.results)

